# revision 3
# baseline (speedup 1.0000x reference)
"""Single-head classical attention on 8 TRN2 NeuronCores.

Problem: B=4, S=2048, D=1024 fp32.
    q = (x @ Wq^T) / sqrt(D); k = x @ Wk^T; v = x @ Wv^T
    out = softmax(q @ k^T) @ v

Sharding: core c handles batch b = c//2 and query-half h = c%2 (1024 query
rows).  K/V are computed over the full 2048 keys of that batch element on
each core (duplicated across the 2 cores sharing a batch) -> no collectives.

Host-side staging: each core receives X^T ([D, S], fp32) for its batch with
the rows *rolled* so its own query half occupies columns 0:1024 (softmax and
P@V are invariant to a consistent permutation of the keys, so rolling rows of
X — which permutes keys of K/V identically — leaves the output unchanged).
This lets a single SPMD program serve all 8 cores.  Weights are passed
pre-transposed ([in, out] layout) since the TensorEngine contracts over the
partition dimension.

On-chip dataflow (all matmuls bf16, fp32 PSUM accumulation):
    K^T[e,s]  = matmul(lhsT=WkT[d,e], rhs=XT[d,s])     contract d
    Q^T[e,m]  = matmul(lhsT=WqT[d,e], rhs=XT[d,0:M])   contract d
    V[s,e]    = matmul(lhsT=XT[d,s],  rhs=WvT[d,e])    contract d
    A^T[s,m]  = matmul(lhsT=K^T[e,s], rhs=Q^T[e,m])    contract e
    P^T[s,m]  = exp(A^T / 32)          (ScalarE, no max-subtraction: logits
                                        are ~N(0,1) so exp cannot overflow)
    Z[1,m]    = matmul(lhsT=ones[s,1], rhs=P^T[s,m])   contract s
    O[m,e]    = matmul(lhsT=P^T[s,m],  rhs=V[s,e])/Z   contract s
"""

import threading

import numpy as np

import concourse.bass as bass
import concourse.tile as tile
from concourse import bacc, mybir
from concourse.bass_utils import run_bass_kernel_spmd

P = 128            # partitions
D = 1024           # embed dim
S = 2048           # seq len (keys per core)
M = 1024           # query rows per core
DT = D // P        # 8  d-tiles  (projection contraction)
ET = D // P        # 8  e-tiles
ST = S // P        # 16 s-tiles
MT = M // P        # 8  m-tiles
NF = 512           # matmul free dim (one fp32 PSUM bank)
SCALE = 1.0 / np.sqrt(np.float32(D))  # 1/32

BF16 = mybir.dt.bfloat16
F32 = mybir.dt.float32


def build_attention_core():
    """Build the SPMD Bass graph for one core (same NEFF on all 8 cores)."""
    nc = bacc.Bacc("TRN2", target_bir_lowering=False, debug=False, num_devices=8)

    xT = nc.dram_tensor("xT", [D, S], F32, kind="ExternalInput")
    wqT = nc.dram_tensor("wqT", [D, D], F32, kind="ExternalInput")
    wkT = nc.dram_tensor("wkT", [D, D], F32, kind="ExternalInput")
    wvT = nc.dram_tensor("wvT", [D, D], F32, kind="ExternalInput")
    out = nc.dram_tensor("out", [M, D], F32, kind="ExternalOutput")

    xT_r = xT.ap().rearrange("(dt p) s -> p dt s", p=P)      # [128, 8, 2048]
    wq_r = wqT.ap().rearrange("(dt p) e -> p dt e", p=P)     # [128, 8, 1024]
    wk_r = wkT.ap().rearrange("(dt p) e -> p dt e", p=P)
    wv_r = wvT.ap().rearrange("(dt p) e -> p dt e", p=P)
    out_r = out.ap().rearrange("(mt p) e -> p mt e", p=P)    # [128, 8, 1024]

    Exp = mybir.ActivationFunctionType.Exp

    with tile.TileContext(nc) as tc:
        with (
            tc.tile_pool(name="persist", bufs=1) as persist,
            tc.tile_pool(name="stage", bufs=3) as stage,
            tc.tile_pool(name="ostage", bufs=3) as ostage,
            tc.tile_pool(name="pp_mm", bufs=2, space="PSUM") as pp_mm,
            tc.tile_pool(name="pp_a", bufs=2, space="PSUM") as pp_a,
            tc.tile_pool(name="pp_z", bufs=2, space="PSUM") as pp_z,
            tc.tile_pool(name="dram", bufs=1, space="DRAM") as dram,
        ):
            # ---- persistent bf16 operands ----
            # xT and pT share one slot: xT dies with the last projection
            # matmul, pT is born in the scores phase.
            xT_bf = persist.tile([P, DT, S], BF16, tag="xp_share", name="xT_bf")
            wq_bf = persist.tile([P, DT, D], BF16, name="wq_bf")
            wk_bf = persist.tile([P, DT, D], BF16, name="wk_bf")
            wv_bf = persist.tile([P, DT, D], BF16, name="wv_bf")
            kT_bf = persist.tile([P, ET, S], BF16, name="kT_bf")
            qT_bf = persist.tile([P, ET, M], BF16, name="qT_bf")
            v_bf = persist.tile([P, ST, D], BF16, name="v_bf")

            ones_bf = persist.tile([P, 1], BF16, name="ones_bf")
            nc.vector.memset(ones_bf[:], 1.0)
            z_row = persist.tile([1, M], F32, name="z_row")
            nc.vector.memset(z_row[:], 0.0)

            # ---- load fp32 inputs, cast to bf16 ----
            def load_cast(dst_bf, src_r, width):
                for dt_i in range(DT):
                    t = stage.tile([P, S], F32, tag="stage")
                    nc.sync.dma_start(t[:, :width], src_r[:, dt_i, :])
                    nc.vector.tensor_copy(dst_bf[:, dt_i, :], t[:, :width])

            load_cast(xT_bf, xT_r, S)
            load_cast(wk_bf, wk_r, D)
            load_cast(wq_bf, wq_r, D)
            load_cast(wv_bf, wv_r, D)

            # ---- projections (contract d over DT k-steps) ----
            # K^T[e, s]
            for et in range(ET):
                for sc in range(S // NF):
                    ps = pp_mm.tile([P, NF], F32, tag="mm")
                    for kt in range(DT):
                        nc.tensor.matmul(
                            ps[:],
                            lhsT=wk_bf[:, kt, et * P:(et + 1) * P],
                            rhs=xT_bf[:, kt, sc * NF:(sc + 1) * NF],
                            start=(kt == 0),
                            stop=(kt == DT - 1),
                        )
                    nc.vector.tensor_copy(kT_bf[:, et, sc * NF:(sc + 1) * NF], ps[:])

            # Q^T[e, m]  (query rows are columns 0:M of xT)
            for et in range(ET):
                for mc in range(M // NF):
                    ps = pp_mm.tile([P, NF], F32, tag="mm")
                    for kt in range(DT):
                        nc.tensor.matmul(
                            ps[:],
                            lhsT=wq_bf[:, kt, et * P:(et + 1) * P],
                            rhs=xT_bf[:, kt, mc * NF:(mc + 1) * NF],
                            start=(kt == 0),
                            stop=(kt == DT - 1),
                        )
                    nc.vector.tensor_copy(qT_bf[:, et, mc * NF:(mc + 1) * NF], ps[:])

            # V[s, e]
            for st in range(ST):
                for ec in range(D // NF):
                    ps = pp_mm.tile([P, NF], F32, tag="mm")
                    for kt in range(DT):
                        nc.tensor.matmul(
                            ps[:],
                            lhsT=xT_bf[:, kt, st * P:(st + 1) * P],
                            rhs=wv_bf[:, kt, ec * NF:(ec + 1) * NF],
                            start=(kt == 0),
                            stop=(kt == DT - 1),
                        )
                    nc.vector.tensor_copy(v_bf[:, st, ec * NF:(ec + 1) * NF], ps[:])

            # ---- scores: A^T = K @ Q^T, P^T = exp(A^T/32), Z += 1s @ P^T ----
            pT_bf = persist.tile([P, ST, M], BF16, tag="xp_share", name="pT_bf")

            for st in range(ST):
                for mc in range(M // NF):
                    ps_a = pp_a.tile([P, NF], F32, tag="a")
                    for et in range(ET):
                        nc.tensor.matmul(
                            ps_a[:],
                            lhsT=kT_bf[:, et, st * P:(st + 1) * P],
                            rhs=qT_bf[:, et, mc * NF:(mc + 1) * NF],
                            start=(et == 0),
                            stop=(et == ET - 1),
                        )
                    nc.scalar.activation(
                        out=pT_bf[:, st, mc * NF:(mc + 1) * NF],
                        in_=ps_a[:],
                        func=Exp,
                        scale=float(SCALE),
                    )
                for mc in range(M // NF):
                    ps_z = pp_z.tile([1, NF], F32, tag="z")
                    nc.tensor.matmul(
                        ps_z[:],
                        lhsT=ones_bf[:],
                        rhs=pT_bf[:, st, mc * NF:(mc + 1) * NF],
                        start=True,
                        stop=True,
                    )
                    nc.vector.tensor_add(
                        out=z_row[:, mc * NF:(mc + 1) * NF],
                        in0=z_row[:, mc * NF:(mc + 1) * NF],
                        in1=ps_z[:],
                    )

            # ---- softmax denominators: [1, M] -> [128, MT] + reciprocal ----
            # partition<->free exchange isn't expressible SBUF->SBUF; bounce
            # the 4KB vector through DRAM where APs are plain byte strides.
            z_dram = dram.tile([1, M], F32, name="z_dram")
            nc.sync.dma_start(z_dram[:], z_row[:])
            z_col = persist.tile([P, MT], F32, name="z_col")
            nc.sync.dma_start(
                z_col[:], z_dram[0, :].rearrange("(t p) -> p t", p=P)
            )
            z_recip = persist.tile([P, MT], F32, name="z_recip")
            nc.vector.reciprocal(z_recip[:], z_col[:])

            # ---- O = (P^T)^T @ V, scaled by 1/Z ----
            for mt in range(MT):
                for ec in range(D // NF):
                    ps_o = pp_mm.tile([P, NF], F32, tag="mm")
                    for st in range(ST):
                        nc.tensor.matmul(
                            ps_o[:],
                            lhsT=pT_bf[:, st, mt * P:(mt + 1) * P],
                            rhs=v_bf[:, st, ec * NF:(ec + 1) * NF],
                            start=(st == 0),
                            stop=(st == ST - 1),
                        )
                    o_t = ostage.tile([P, NF], F32, tag="o")
                    nc.vector.tensor_scalar_mul(
                        o_t[:], ps_o[:], z_recip[:, mt:mt + 1]
                    )
                    nc.sync.dma_start(out_r[:, mt, ec * NF:(ec + 1) * NF], o_t[:])

    nc.compile()
    return nc


_nc_lock = threading.Lock()
_nc_cache = []


def _get_nc():
    with _nc_lock:
        if not _nc_cache:
            _nc_cache.append(build_attention_core())
        return _nc_cache[0]


def _make_in_maps(inputs, w_q, w_k, w_v):
    wqT = np.ascontiguousarray(np.asarray(w_q, dtype=np.float32).T)
    wkT = np.ascontiguousarray(np.asarray(w_k, dtype=np.float32).T)
    wvT = np.ascontiguousarray(np.asarray(w_v, dtype=np.float32).T)
    in_maps = []
    for core in range(8):
        b, half = core // 2, core % 2
        xb = np.asarray(inputs[b], dtype=np.float32)
        if half:
            xb = np.roll(xb, -half * M, axis=0)
        in_maps.append(
            {
                "xT": np.ascontiguousarray(xb.T),
                "wqT": wqT,
                "wkT": wkT,
                "wvT": wvT,
            }
        )
    return in_maps


def run(inputs, w_q, w_k, w_v, **run_kwargs):
    """Run the 8-core SPMD kernel; returns (full_output, BassKernelResults)."""
    nc = _get_nc()
    in_maps = _make_in_maps(inputs, w_q, w_k, w_v)
    res = run_bass_kernel_spmd(nc, in_maps, core_ids=list(range(8)), **run_kwargs)
    full = np.empty((4, S, D), dtype=np.float32)
    for core in range(8):
        b, half = core // 2, core % 2
        full[b, half * M:(half + 1) * M, :] = res.results[core]["out"]
    return full, res


def kernel(**inputs) -> np.ndarray:
    out, _ = run(inputs["inputs"], inputs["w_q"], inputs["w_k"], inputs["w_v"])
    return out


# revision 6
# speedup vs baseline: 1.2723x; 1.2723x over previous
"""Single-head classical attention on 8 TRN2 NeuronCores.

Problem: B=4, S=2048, D=1024 fp32.
    q = (x @ Wq^T) / sqrt(D); k = x @ Wk^T; v = x @ Wv^T
    out = softmax(q @ k^T) @ v

Sharding: core c handles batch b = c//2 and query-half h = c%2 (1024 query
rows).  K/V are computed over the full 2048 keys of that batch element on
each core (duplicated across the 2 cores sharing a batch) -> no collectives.

Host-side staging: each core receives X^T ([D, S], fp32) for its batch with
the rows *rolled* so its own query half occupies columns 0:1024 (softmax and
P@V are invariant to a consistent permutation of the keys, so rolling rows of
X — which permutes keys of K/V identically — leaves the output unchanged).
This lets a single SPMD program serve all 8 cores.  Weights are passed
pre-transposed ([in, out] layout) since the TensorEngine contracts over the
partition dimension.

On-chip dataflow (all matmuls bf16, fp32 PSUM accumulation):
    K^T[e,s]  = matmul(lhsT=WkT[d,e], rhs=XT[d,s])     contract d
    Q^T[e,m]  = matmul(lhsT=WqT[d,e], rhs=XT[d,0:M])   contract d
    V[s,e]    = matmul(lhsT=XT[d,s],  rhs=WvT[d,e])    contract d
    A^T[s,m]  = matmul(lhsT=K^T[e,s], rhs=Q^T[e,m])    contract e
    P^T[s,m]  = exp(A^T / 32)          (ScalarE, no max-subtraction: logits
                                        are ~N(0,1) so exp cannot overflow)
    Z[1,m]    = matmul(lhsT=ones[s,1], rhs=P^T[s,m])   contract s
    O[m,e]    = matmul(lhsT=P^T[s,m],  rhs=V[s,e])/Z   contract s
"""

import threading

import numpy as np

import concourse.bass as bass
import concourse.tile as tile
from concourse import bacc, mybir
from concourse.bass_utils import run_bass_kernel_spmd

P = 128            # partitions
D = 1024           # embed dim
S = 2048           # seq len (keys per core)
M = 1024           # query rows per core
DT = D // P        # 8  d-tiles  (projection contraction)
ET = D // P        # 8  e-tiles
ST = S // P        # 16 s-tiles
MT = M // P        # 8  m-tiles
NF = 512           # matmul free dim (one fp32 PSUM bank)
SCALE = 1.0 / np.sqrt(np.float32(D))  # 1/32

BF16 = mybir.dt.bfloat16
F32 = mybir.dt.float32


def build_attention_core():
    """Build the SPMD Bass graph for one core (same NEFF on all 8 cores)."""
    nc = bacc.Bacc("TRN2", target_bir_lowering=False, debug=False, num_devices=8)

    xT = nc.dram_tensor("xT", [D, S], F32, kind="ExternalInput")
    wqT = nc.dram_tensor("wqT", [D, D], F32, kind="ExternalInput")
    wkT = nc.dram_tensor("wkT", [D, D], F32, kind="ExternalInput")
    wvT = nc.dram_tensor("wvT", [D, D], F32, kind="ExternalInput")
    out = nc.dram_tensor("out", [M, D], F32, kind="ExternalOutput")

    xT_r = xT.ap().rearrange("(dt p) s -> p dt s", p=P)      # [128, 8, 2048]
    wq_r = wqT.ap().rearrange("(dt p) e -> p dt e", p=P)     # [128, 8, 1024]
    wk_r = wkT.ap().rearrange("(dt p) e -> p dt e", p=P)
    wv_r = wvT.ap().rearrange("(dt p) e -> p dt e", p=P)
    out_r = out.ap().rearrange("(mt p) e -> p mt e", p=P)    # [128, 8, 1024]

    Exp = mybir.ActivationFunctionType.Exp

    with tile.TileContext(nc) as tc:
        with (
            tc.tile_pool(name="persist", bufs=1) as persist,
            tc.tile_pool(name="stage", bufs=4) as stage,
            tc.tile_pool(name="ostage", bufs=3) as ostage,
            tc.tile_pool(name="pp_mm", bufs=6, space="PSUM") as pp_mm,
            tc.tile_pool(name="pp_z", bufs=2, space="PSUM") as pp_z,
            tc.tile_pool(name="dram", bufs=1, space="DRAM") as dram,
        ):
            pp_a = pp_mm
            # ---- persistent bf16 operands ----
            # xT and pT share one slot: xT dies with the last projection
            # matmul, pT is born in the scores phase.
            xT_bf = persist.tile([P, DT, S], BF16, tag="xp_share", name="xT_bf")
            wq_bf = persist.tile([P, DT, D], BF16, name="wq_bf")
            wk_bf = persist.tile([P, DT, D], BF16, name="wk_bf")
            wv_bf = persist.tile([P, DT, D], BF16, name="wv_bf")
            kT_bf = persist.tile([P, ET, S], BF16, name="kT_bf")
            qT_bf = persist.tile([P, ET, M], BF16, name="qT_bf")
            v_bf = persist.tile([P, ST, D], BF16, name="v_bf")

            ones_bf = persist.tile([P, 1], BF16, name="ones_bf")
            nc.vector.memset(ones_bf[:], 1.0)
            z_row = persist.tile([1, M], F32, name="z_row")
            nc.vector.memset(z_row[:], 0.0)

            # ---- load fp32 inputs, cast to bf16 ----
            # Emission order = DMA queue order = arrival order.  The PE's
            # first work is Q^T (needs wq + query half of xT), so stream
            # those first, then wk + the key half of xT (K^T), then wv (V).
            def load_cast_cols(dst_bf, src_r, dt_i, c0, c1):
                t = stage.tile([P, M], F32, tag="stage")
                nc.sync.dma_start(t[:, : c1 - c0], src_r[:, dt_i, c0:c1])
                nc.vector.tensor_copy(dst_bf[:, dt_i, c0:c1], t[:, : c1 - c0])

            for kt in range(DT):
                load_cast_cols(wq_bf, wq_r, kt, 0, D)
                load_cast_cols(xT_bf, xT_r, kt, 0, M)
            for kt in range(DT):
                load_cast_cols(wk_bf, wk_r, kt, 0, D)
                load_cast_cols(xT_bf, xT_r, kt, M, S)
            for kt in range(DT):
                load_cast_cols(wv_bf, wv_r, kt, 0, D)

            # ---- projections (contract d over DT k-steps) ----
            # Q^T[e, m]  (query rows are columns 0:M of xT)
            for et in range(ET):
                for mc in range(M // NF):
                    ps = pp_mm.tile([P, NF], F32, tag="mm")
                    for kt in range(DT):
                        nc.tensor.matmul(
                            ps[:],
                            lhsT=wq_bf[:, kt, et * P:(et + 1) * P],
                            rhs=xT_bf[:, kt, mc * NF:(mc + 1) * NF],
                            start=(kt == 0),
                            stop=(kt == DT - 1),
                        )
                    nc.vector.tensor_copy(qT_bf[:, et, mc * NF:(mc + 1) * NF], ps[:])

            # K^T[e, s]  (sc-outer so A^T s-tiles unlock per column block)
            for sc in range(S // NF):
                for et in range(ET):
                    ps = pp_mm.tile([P, NF], F32, tag="mm")
                    for kt in range(DT):
                        nc.tensor.matmul(
                            ps[:],
                            lhsT=wk_bf[:, kt, et * P:(et + 1) * P],
                            rhs=xT_bf[:, kt, sc * NF:(sc + 1) * NF],
                            start=(kt == 0),
                            stop=(kt == DT - 1),
                        )
                    nc.vector.tensor_copy(kT_bf[:, et, sc * NF:(sc + 1) * NF], ps[:])

            # V[s, e]
            for st in range(ST):
                for ec in range(D // NF):
                    ps = pp_mm.tile([P, NF], F32, tag="mm")
                    for kt in range(DT):
                        nc.tensor.matmul(
                            ps[:],
                            lhsT=xT_bf[:, kt, st * P:(st + 1) * P],
                            rhs=wv_bf[:, kt, ec * NF:(ec + 1) * NF],
                            start=(kt == 0),
                            stop=(kt == DT - 1),
                        )
                    nc.vector.tensor_copy(v_bf[:, st, ec * NF:(ec + 1) * NF], ps[:])

            # ---- scores: A^T = K @ Q^T, P^T = exp(A^T/32), Z += 1s @ P^T ----
            pT_bf = persist.tile([P, ST, M], BF16, tag="xp_share", name="pT_bf")

            for st in range(ST):
                for mc in range(M // NF):
                    ps_a = pp_a.tile([P, NF], F32, tag="mm")
                    for et in range(ET):
                        nc.tensor.matmul(
                            ps_a[:],
                            lhsT=kT_bf[:, et, st * P:(st + 1) * P],
                            rhs=qT_bf[:, et, mc * NF:(mc + 1) * NF],
                            start=(et == 0),
                            stop=(et == ET - 1),
                        )
                    nc.scalar.activation(
                        out=pT_bf[:, st, mc * NF:(mc + 1) * NF],
                        in_=ps_a[:],
                        func=Exp,
                        scale=float(SCALE),
                    )
                for mc in range(M // NF):
                    ps_z = pp_z.tile([1, NF], F32, tag="z")
                    nc.tensor.matmul(
                        ps_z[:],
                        lhsT=ones_bf[:],
                        rhs=pT_bf[:, st, mc * NF:(mc + 1) * NF],
                        start=True,
                        stop=True,
                    )
                    nc.vector.tensor_add(
                        out=z_row[:, mc * NF:(mc + 1) * NF],
                        in0=z_row[:, mc * NF:(mc + 1) * NF],
                        in1=ps_z[:],
                    )

            # ---- softmax denominators: [1, M] -> [128, MT] + reciprocal ----
            # partition<->free exchange isn't expressible SBUF->SBUF; bounce
            # the 4KB vector through DRAM where APs are plain byte strides.
            z_dram = dram.tile([1, M], F32, name="z_dram")
            nc.sync.dma_start(z_dram[:], z_row[:])
            z_col = persist.tile([P, MT], F32, name="z_col")
            nc.sync.dma_start(
                z_col[:], z_dram[0, :].rearrange("(t p) -> p t", p=P)
            )
            z_recip = persist.tile([P, MT], F32, name="z_recip")
            nc.vector.reciprocal(z_recip[:], z_col[:])

            # ---- O = (P^T)^T @ V, scaled by 1/Z ----
            for mt in range(MT):
                for ec in range(D // NF):
                    ps_o = pp_mm.tile([P, NF], F32, tag="mm")
                    for st in range(ST):
                        nc.tensor.matmul(
                            ps_o[:],
                            lhsT=pT_bf[:, st, mt * P:(mt + 1) * P],
                            rhs=v_bf[:, st, ec * NF:(ec + 1) * NF],
                            start=(st == 0),
                            stop=(st == ST - 1),
                        )
                    o_t = ostage.tile([P, NF], F32, tag="o")
                    nc.vector.tensor_scalar_mul(
                        o_t[:], ps_o[:], z_recip[:, mt:mt + 1]
                    )
                    nc.sync.dma_start(out_r[:, mt, ec * NF:(ec + 1) * NF], o_t[:])

    nc.compile()
    return nc


_nc_lock = threading.Lock()
_nc_cache = []


def _get_nc():
    with _nc_lock:
        if not _nc_cache:
            _nc_cache.append(build_attention_core())
        return _nc_cache[0]


def _make_in_maps(inputs, w_q, w_k, w_v):
    wqT = np.ascontiguousarray(np.asarray(w_q, dtype=np.float32).T)
    wkT = np.ascontiguousarray(np.asarray(w_k, dtype=np.float32).T)
    wvT = np.ascontiguousarray(np.asarray(w_v, dtype=np.float32).T)
    in_maps = []
    for core in range(8):
        b, half = core // 2, core % 2
        xb = np.asarray(inputs[b], dtype=np.float32)
        if half:
            xb = np.roll(xb, -half * M, axis=0)
        in_maps.append(
            {
                "xT": np.ascontiguousarray(xb.T),
                "wqT": wqT,
                "wkT": wkT,
                "wvT": wvT,
            }
        )
    return in_maps


def run(inputs, w_q, w_k, w_v, **run_kwargs):
    """Run the 8-core SPMD kernel; returns (full_output, BassKernelResults)."""
    nc = _get_nc()
    in_maps = _make_in_maps(inputs, w_q, w_k, w_v)
    res = run_bass_kernel_spmd(nc, in_maps, core_ids=list(range(8)), **run_kwargs)
    full = np.empty((4, S, D), dtype=np.float32)
    for core in range(8):
        b, half = core // 2, core % 2
        full[b, half * M:(half + 1) * M, :] = res.results[core]["out"]
    return full, res


def kernel(**inputs) -> np.ndarray:
    out, _ = run(inputs["inputs"], inputs["w_q"], inputs["w_k"], inputs["w_v"])
    return out
